# revision 8
# baseline (speedup 1.0000x reference)
"""Multi-head attention (B=2, N=2048, D=1024, H=16, hd=64) on 8 trn2 NeuronCores.

Sharding: 8 cores = 2 (batch) x 4 (head groups of 4 heads).
Core c: batch b = c // 4, heads hg*4 .. hg*4+3 where hg = c % 4.

Per-core program (identical SPMD program, per-core data):
  inputs (DRAM):
    xT     [1024, 2048]  = x[b].T
    wqkT   [1024, 512]   = w_qkv[[q rows, k rows] of local heads].T
    wvT    [1024, 256]   = w_qkv[v rows of local heads].T
    wprojT [256, 1024]   = w_proj[:, local head cols].T
  output:
    out    [2048, 1024]  partial (row-parallel) projection output

  1) qkT  [512, 2048]  = wqkT.T @ xT        (q,k in transposed layout)
     v    [2048, 256]  natural layout, stored per head with 64 ones
     columns appended (v_sb[:, st, h, 64:128] = 1) so the PV matmul
     replicates the softmax denominator across partitions 64..127.
  2) per (head, q-block 512): scores_T [keys, q] = kT.T @ qT, exp on ACT
     (scale 1/8 fused) -> probs bf16; PV with ones-augmented V gives
     psum [128, 512]: rows 0..63 unnormalized out_T, rows 64..127 the
     denominator replicated. reciprocal_approx_fast (DVE custom op) +
     one cross-partition tensor_mul normalizes straight into proj
     layout (ao_sb) - no PE broadcast matmul, no DMA hop.
  3) partial = attn_out_T.T @ wprojT -> [2048, 1024], DMA'd to DRAM
     directly from PSUM.

Host unshard: out[b] = sum over 4 head-group partials + b_proj.
"""

import sys

if "/opt/trn_rl_repo" not in sys.path:
    sys.path.insert(0, "/opt/trn_rl_repo")

import numpy as np

B, N, D, H, HD = 2, 2048, 1024, 16, 64
NCORES = 8
HPC = 4               # heads per core
LQK = HPC * HD        # 256 local q (or k) rows
SCALE = HD ** -0.5    # 0.125

_CACHE = {}


def _emit_body(tc, out_d, xT_d, wqkT_d, wvT_d, wprojT_d):
    from concourse import mybir

    F32 = mybir.dt.float32
    BF16 = mybir.dt.bfloat16
    Exp = mybir.ActivationFunctionType.Exp

    nc = tc.nc
    KT = D // 128        # 8 contraction tiles for qkv gemms
    NB = N // 512        # 4 seq blocks
    NT = N // 128        # 16 seq tiles

    if True:
        with (
            nc.allow_low_precision(reason="bf16 matmul operands"),
            tc.tile_pool(name="w", bufs=1) as wpool,
            tc.tile_pool(name="x", bufs=1) as xpool,
            tc.tile_pool(name="qk", bufs=1) as qkpool,
            tc.tile_pool(name="vaug", bufs=1) as vapool,
            tc.tile_pool(name="ao", bufs=1) as aopool,
            tc.tile_pool(name="probs", bufs=3) as prpool,
            tc.tile_pool(name="recip", bufs=2) as rcpool,
            tc.tile_pool(name="stage", bufs=2) as stpool,
            tc.tile_pool(name="sc", bufs=2, space="PSUM") as scpool,
            tc.tile_pool(name="small", bufs=2, space="PSUM") as smps,
            tc.tile_pool(name="pv", bufs=2, space="PSUM") as pvps,
        ):
            # ---- SBUF tiles ----
            x_sb = xpool.tile([128, KT, N], BF16)
            wqk_sb = wpool.tile([128, KT, 2 * LQK], BF16)
            wv_sb = wpool.tile([128, KT, LQK], BF16)
            wproj_sb = wpool.tile([128, 2, D], BF16)
            qk_sb = qkpool.tile([128, 4, N], BF16)
            v_sb = vapool.tile([128, NT, HPC, 128], BF16)
            ao_sb = aopool.tile([128, 2, N], BF16)

            # ones columns so PV replicates the denominator on rows 0..63
            # (custom-DVE recip requires its input at partition offset 0)
            nc.gpsimd.memset(v_sb[:, :, :, 0:HD], 1.0)

            # ---- input DMAs, chunked so compute starts early ----
            xT_r = xT_d.rearrange("(kt p) n -> p kt n", p=128)
            wqkT_r = wqkT_d.rearrange("(kt p) m -> p kt m", p=128)
            wvT_r = wvT_d.rearrange("(kt p) m -> p kt m", p=128)
            for kt in range(KT):
                nc.sync.dma_start(out=wqk_sb[:, kt, :], in_=wqkT_r[:, kt, :])
            for kt in range(KT):
                nc.sync.dma_start(out=x_sb[:, kt, 0:1024], in_=xT_r[:, kt, 0:1024])
            for kt in range(KT):
                nc.sync.dma_start(out=wv_sb[:, kt, :], in_=wvT_r[:, kt, :])
            for kt in range(KT):
                nc.sync.dma_start(out=x_sb[:, kt, 1024:2048],
                                  in_=xT_r[:, kt, 1024:2048])
            nc.sync.dma_start(
                out=wproj_sb[:, :, :],
                in_=wprojT_d.rearrange("(kt p) o -> p kt o", p=128))

            # ---- qkT = wqkT.T @ xT : [512, 2048] ----
            # qk_sb m-tile layout: m=0: q heads 0,1 / m=1: q heads 2,3
            #                      m=2: k heads 0,1 / m=3: k heads 2,3
            def qk_gemm(m):
                wcol = (m % 2) * 128 + (0 if m < 2 else 2 * LQK // 2)
                for nb in range(NB):
                    ps = smps.tile([128, 512], F32, tag="small")
                    for kt in range(KT):
                        nc.tensor.matmul(
                            ps[:, :],
                            wqk_sb[:, kt, wcol:wcol + 128],
                            x_sb[:, kt, nb * 512:(nb + 1) * 512],
                            start=(kt == 0), stop=(kt == KT - 1),
                        )
                    nc.vector.tensor_copy(
                        qk_sb[:, m, nb * 512:(nb + 1) * 512], ps[:, :])

            # ---- v natural [2048, 256]: st pairs -> [128, st, h, 0:64] ----
            def v_gemm(g):               # g = 0..7, st pair (2g, 2g+1)
                ps = smps.tile([128, 512], F32, tag="small")
                for i in range(2):
                    st = 2 * g + i
                    for kt in range(KT):
                        nc.tensor.matmul(
                            ps[:, i * 256:(i + 1) * 256],
                            x_sb[:, kt, st * 128:(st + 1) * 128],
                            wv_sb[:, kt, :],
                            start=(kt == 0), stop=(kt == KT - 1),
                        )
                nc.vector.tensor_copy(
                    v_sb[:, 2 * g:2 * g + 2, :, HD:128],
                    ps[:, :].rearrange("p (s h d) -> p s h d", s=2, h=HPC))

            for m in (2, 0):
                qk_gemm(m)
            for g in range(NT // 2):
                v_gemm(g)
            for m in (3, 1):
                qk_gemm(m)

            # ---- attention: scores -> exp -> PV -> normalize ----
            def attn_chain(h, qb):
                pi = (h % 2) * 64
                mq, mk = h // 2, 2 + h // 2
                qT = qk_sb[pi:pi + 64, mq, qb * 512:(qb + 1) * 512]
                pv = pvps.tile([128, 512], F32, tag="pv")
                for kk in range(KT):          # pairs of key tiles
                    sc = scpool.tile([128, 1024], F32, tag="sc")
                    pr = prpool.tile([128, 1024], BF16, tag="probs")
                    for j in range(2):
                        kt = 2 * kk + j
                        kT = qk_sb[pi:pi + 64, mk, kt * 128:(kt + 1) * 128]
                        nc.tensor.matmul(
                            sc[:, j * 512:(j + 1) * 512], kT, qT,
                            start=True, stop=True)
                    nc.scalar.activation(pr[:, :], sc[:, :], Exp, scale=SCALE)
                    for j in range(2):
                        kt = 2 * kk + j
                        nc.tensor.matmul(
                            pv[:, :],
                            v_sb[:, kt, h, :],
                            pr[:, j * 512:(j + 1) * 512],
                            start=(kk == 0 and j == 0),
                            stop=(kk == KT - 1 and j == 1),
                        )
                recip = rcpool.tile([64, 512], F32, tag="recip")
                nc.vector.reciprocal_approx_fast(recip[:, :], pv[0:64, :])
                nc.vector.tensor_mul(
                    ao_sb[pi:pi + 64, h // 2, qb * 512:(qb + 1) * 512],
                    pv[64:128, :], recip[:, :])

            def proj(qb):
                for nt in range(qb * 4, qb * 4 + 4):
                    outst = stpool.tile([128, D], F32, tag="outst")
                    for ob in range(2):
                        ps = smps.tile([128, 512], F32, tag="small")
                        for kt2 in range(2):
                            nc.tensor.matmul(
                                ps[:, :],
                                ao_sb[:, kt2, nt * 128:(nt + 1) * 128],
                                wproj_sb[:, kt2, ob * 512:(ob + 1) * 512],
                                start=(kt2 == 0), stop=(kt2 == 1),
                            )
                        nc.vector.tensor_copy(
                            outst[:, ob * 512:(ob + 1) * 512], ps[:, :])
                    nc.sync.dma_start(
                        out=out_d[nt * 128:(nt + 1) * 128, :], in_=outst[:, :])

            for qb in range(NB):
                for h in range(HPC):
                    attn_chain(h, qb)
                proj(qb)


def _build_program():
    import concourse.tile as tile
    from concourse import bacc, mybir

    F32 = mybir.dt.float32
    BF16 = mybir.dt.bfloat16

    nc = bacc.Bacc("TRN2", target_bir_lowering=False, debug=False,
                   num_devices=NCORES)

    xT_d = nc.dram_tensor("xT", [D, N], BF16, kind="ExternalInput").ap()
    wqkT_d = nc.dram_tensor("wqkT", [D, 2 * LQK], BF16, kind="ExternalInput").ap()
    wvT_d = nc.dram_tensor("wvT", [D, LQK], BF16, kind="ExternalInput").ap()
    wprojT_d = nc.dram_tensor("wprojT", [LQK, D], BF16, kind="ExternalInput").ap()
    out_d = nc.dram_tensor("out", [N, D], F32, kind="ExternalOutput").ap()

    with tile.TileContext(nc) as tc:
        _emit_body(tc, out_d, xT_d, wqkT_d, wvT_d, wprojT_d)

    nc.compile()
    return nc


def _get_program():
    if "nc" not in _CACHE:
        _CACHE["nc"] = _build_program()
    return _CACHE["nc"]


def _make_in_maps(x, w_qkv, w_proj):
    import ml_dtypes
    bf16 = ml_dtypes.bfloat16
    x = np.asarray(x, dtype=np.float32)
    w_qkv = np.asarray(w_qkv, dtype=np.float32)
    w_proj = np.asarray(w_proj, dtype=np.float32)
    xT = [np.ascontiguousarray(x[b].T).astype(bf16) for b in range(B)]
    in_maps = []
    for c in range(NCORES):
        b, hg = c // 4, c % 4
        rows = slice(hg * LQK, (hg + 1) * LQK)
        qk_rows = np.r_[np.arange(hg * LQK, (hg + 1) * LQK),
                        D + np.arange(hg * LQK, (hg + 1) * LQK)]
        in_maps.append({
            "xT": xT[b],
            "wqkT": np.ascontiguousarray(w_qkv[qk_rows, :].T).astype(bf16),
            "wvT": np.ascontiguousarray(
                w_qkv[2 * D + np.arange(hg * LQK, (hg + 1) * LQK), :].T).astype(bf16),
            "wprojT": np.ascontiguousarray(w_proj[:, rows].T).astype(bf16),
        })
    return in_maps


def kernel(x, w_qkv, w_proj, b_proj, _return_results=False, _trace=False):
    from concourse import bass_utils

    nc = _get_program()
    in_maps = _make_in_maps(x, w_qkv, w_proj)
    res = bass_utils.run_bass_kernel_spmd(
        nc, in_maps, list(range(NCORES)), trace=_trace)
    partials = np.stack([res.results[c]["out"] for c in range(NCORES)])
    out = partials.reshape(B, 4, N, D).sum(axis=1, dtype=np.float32)
    out = out + np.asarray(b_proj, dtype=np.float32)[None, None, :]
    out = out.astype(np.float32)
    if _return_results:
        return out, res
    return out


# revision 11
# speedup vs baseline: 1.2343x; 1.2343x over previous
"""Multi-head attention (B=2, N=2048, D=1024, H=16, hd=64) on 8 trn2 NeuronCores.

Sharding: 8 cores = 2 (batch) x 4 (head groups of 4 heads).
Core c: batch b = c // 4, heads hg*4 .. hg*4+3 where hg = c % 4.

Per-core program (identical SPMD program, per-core data):
  inputs (DRAM):
    xT     [1024, 2048]  = x[b].T
    wqkT   [1024, 512]   = w_qkv[[q rows, k rows] of local heads].T
    wvT    [1024, 256]   = w_qkv[v rows of local heads].T
    wprojT [256, 1024]   = w_proj[:, local head cols].T
  output:
    out    [2048, 1024]  partial (row-parallel) projection output

  1) qkT  [512, 2048]  = wqkT.T @ xT        (q,k in transposed layout)
     v    [2048, 256]  natural layout, stored per head with 64 ones
     columns appended (v_sb[:, st, h, 64:128] = 1) so the PV matmul
     replicates the softmax denominator across partitions 64..127.
  2) per (head, q-block 512): scores_T [keys, q] = kT.T @ qT, exp on ACT
     (scale 1/8 fused) -> probs bf16; PV with ones-augmented V gives
     psum [128, 512]: rows 0..63 unnormalized out_T, rows 64..127 the
     denominator replicated. reciprocal_approx_fast (DVE custom op) +
     one cross-partition tensor_mul normalizes straight into proj
     layout (ao_sb) - no PE broadcast matmul, no DMA hop.
  3) partial = attn_out_T.T @ wprojT -> [2048, 1024], DMA'd to DRAM
     directly from PSUM.

Host unshard: out[b] = sum over 4 head-group partials + b_proj.
"""

import sys

if "/opt/trn_rl_repo" not in sys.path:
    sys.path.insert(0, "/opt/trn_rl_repo")

import numpy as np

B, N, D, H, HD = 2, 2048, 1024, 16, 64
NCORES = 8
HPC = 4               # heads per core
LQK = HPC * HD        # 256 local q (or k) rows
SCALE = HD ** -0.5    # 0.125

_CACHE = {}


def _emit_body(tc, out_d, xT_d, wqkT_d, wvT_d, wprojT_d):
    from concourse import mybir

    F32 = mybir.dt.float32
    BF16 = mybir.dt.bfloat16
    Exp = mybir.ActivationFunctionType.Exp

    nc = tc.nc
    KT = D // 128        # 8 contraction tiles for qkv gemms
    NB = N // 512        # 4 seq blocks
    NT = N // 128        # 16 seq tiles

    if True:
        with (
            nc.allow_low_precision(reason="bf16 matmul operands"),
            tc.tile_pool(name="w", bufs=1) as wpool,
            tc.tile_pool(name="x", bufs=1) as xpool,
            tc.tile_pool(name="qk", bufs=1) as qkpool,
            tc.tile_pool(name="vaug", bufs=1) as vapool,
            tc.tile_pool(name="ao", bufs=1) as aopool,
            tc.tile_pool(name="probs", bufs=3) as prpool,
            tc.tile_pool(name="recip", bufs=2) as rcpool,
            tc.tile_pool(name="stage", bufs=2) as stpool,
            tc.tile_pool(name="sc", bufs=2, space="PSUM") as scpool,
            tc.tile_pool(name="small", bufs=2, space="PSUM") as smps,
            tc.tile_pool(name="pv", bufs=2, space="PSUM") as pvps,
        ):
            # ---- SBUF tiles ----
            x_sb = xpool.tile([128, KT, N], BF16)
            wqk_sb = wpool.tile([128, KT, 2 * LQK], BF16)
            wv_sb = wpool.tile([128, KT, LQK], BF16)
            wproj_sb = wpool.tile([128, 2, D], BF16)
            qk_sb = qkpool.tile([128, 4, N], BF16)
            v_sb = vapool.tile([128, NT, HPC, 128], BF16)
            ao_sb = aopool.tile([128, 2, N], BF16)

            # ones columns so PV replicates the denominator on rows 0..63
            # (custom-DVE recip requires its input at partition offset 0)
            nc.gpsimd.memset(v_sb[:, :, :, 0:HD], 1.0)

            # ---- input DMAs, chunked so compute starts early ----
            xT_r = xT_d.rearrange("(kt p) n -> p kt n", p=128)
            wqkT_r = wqkT_d.rearrange("(kt p) m -> p kt m", p=128)
            wvT_r = wvT_d.rearrange("(kt p) m -> p kt m", p=128)
            for kt in range(KT):
                nc.sync.dma_start(out=wqk_sb[:, kt, :], in_=wqkT_r[:, kt, :])
            for kt in range(KT):
                nc.sync.dma_start(out=x_sb[:, kt, 0:1024], in_=xT_r[:, kt, 0:1024])
            for kt in range(KT):
                nc.sync.dma_start(out=wv_sb[:, kt, :], in_=wvT_r[:, kt, :])
            for kt in range(KT):
                nc.sync.dma_start(out=x_sb[:, kt, 1024:2048],
                                  in_=xT_r[:, kt, 1024:2048])
            nc.sync.dma_start(
                out=wproj_sb[:, :, :],
                in_=wprojT_d.rearrange("(kt p) o -> p kt o", p=128))

            # ---- qkT = wqkT.T @ xT : [512, 2048] ----
            # qk_sb m-tile layout: m=0: q heads 0,1 / m=1: q heads 2,3
            #                      m=2: k heads 0,1 / m=3: k heads 2,3
            def qk_gemm(m, nbs):
                wcol = (m % 2) * 128 + (0 if m < 2 else 2 * LQK // 2)
                for nb in nbs:
                    ps = smps.tile([128, 512], F32, tag="small")
                    for kt in range(KT):
                        nc.tensor.matmul(
                            ps[:, :],
                            wqk_sb[:, kt, wcol:wcol + 128],
                            x_sb[:, kt, nb * 512:(nb + 1) * 512],
                            start=(kt == 0), stop=(kt == KT - 1),
                        )
                    nc.vector.tensor_copy(
                        qk_sb[:, m, nb * 512:(nb + 1) * 512], ps[:, :])

            # ---- v natural [2048, 256]: st pairs -> [128, st, h, 0:64] ----
            def v_gemm(g):               # g = 0..7, st pair (2g, 2g+1)
                ps = smps.tile([128, 512], F32, tag="small")
                for i in range(2):
                    st = 2 * g + i
                    for kt in range(KT):
                        nc.tensor.matmul(
                            ps[:, i * 256:(i + 1) * 256],
                            x_sb[:, kt, st * 128:(st + 1) * 128],
                            wv_sb[:, kt, :],
                            start=(kt == 0), stop=(kt == KT - 1),
                        )
                nc.vector.tensor_copy(
                    v_sb[:, 2 * g:2 * g + 2, :, HD:128],
                    ps[:, :].rearrange("p (s h d) -> p s h d", s=2, h=HPC))

            # critical prefix: just enough for the first attention chains
            qk_gemm(2, range(NB))
            qk_gemm(0, [0])
            for g in range(5):
                v_gemm(g)

            # ---- attention: scores -> exp -> PV -> normalize ----
            def attn_chain(h, qb):
                pi = (h % 2) * 64
                mq, mk = h // 2, 2 + h // 2
                qT = qk_sb[pi:pi + 64, mq, qb * 512:(qb + 1) * 512]
                pv = pvps.tile([128, 512], F32, tag="pv")
                for kk in range(KT):          # pairs of key tiles
                    sc = scpool.tile([128, 1024], F32, tag="sc")
                    pr = prpool.tile([128, 1024], BF16, tag="probs")
                    for j in range(2):
                        kt = 2 * kk + j
                        kT = qk_sb[pi:pi + 64, mk, kt * 128:(kt + 1) * 128]
                        nc.tensor.matmul(
                            sc[:, j * 512:(j + 1) * 512], kT, qT,
                            start=True, stop=True)
                    nc.scalar.activation(pr[:, :], sc[:, :], Exp, scale=SCALE)
                    for j in range(2):
                        kt = 2 * kk + j
                        nc.tensor.matmul(
                            pv[:, :],
                            v_sb[:, kt, h, :],
                            pr[:, j * 512:(j + 1) * 512],
                            start=(kk == 0 and j == 0),
                            stop=(kk == KT - 1 and j == 1),
                        )
                recip = rcpool.tile([64, 512], F32, tag="recip")
                nc.vector.reciprocal_approx_fast(recip[:, :], pv[0:64, :])
                nc.vector.tensor_mul(
                    ao_sb[pi:pi + 64, h // 2, qb * 512:(qb + 1) * 512],
                    pv[64:128, :], recip[:, :])

            def proj(qb):
                for nt in range(qb * 4, qb * 4 + 4):
                    outst = stpool.tile([128, D], F32, tag="outst")
                    for ob in range(2):
                        ps = smps.tile([128, 512], F32, tag="small")
                        for kt2 in range(2):
                            nc.tensor.matmul(
                                ps[:, :],
                                ao_sb[:, kt2, nt * 128:(nt + 1) * 128],
                                wproj_sb[:, kt2, ob * 512:(ob + 1) * 512],
                                start=(kt2 == 0), stop=(kt2 == 1),
                            )
                        nc.vector.tensor_copy(
                            outst[:, ob * 512:(ob + 1) * 512], ps[:, :])
                    nc.sync.dma_start(
                        out=out_d[nt * 128:(nt + 1) * 128, :], in_=outst[:, :])

            # remaining gemm work: deprioritized PE filler that slots into
            # the gaps of the ACT(exp)-paced attention chains
            with tc.high_priority(offset=-1000000):
                for g in range(5, NT // 2):
                    v_gemm(g)
                qk_gemm(3, range(NB))
                qk_gemm(1, [0])
                qk_gemm(0, [1, 2, 3])
                qk_gemm(1, [1, 2, 3])

            for qb in range(NB):
                for h in range(HPC):
                    attn_chain(h, qb)
                proj(qb)


def _build_program():
    import concourse.tile as tile
    from concourse import bacc, mybir

    F32 = mybir.dt.float32
    BF16 = mybir.dt.bfloat16

    nc = bacc.Bacc("TRN2", target_bir_lowering=False, debug=False,
                   num_devices=NCORES)

    xT_d = nc.dram_tensor("xT", [D, N], BF16, kind="ExternalInput").ap()
    wqkT_d = nc.dram_tensor("wqkT", [D, 2 * LQK], BF16, kind="ExternalInput").ap()
    wvT_d = nc.dram_tensor("wvT", [D, LQK], BF16, kind="ExternalInput").ap()
    wprojT_d = nc.dram_tensor("wprojT", [LQK, D], BF16, kind="ExternalInput").ap()
    out_d = nc.dram_tensor("out", [N, D], F32, kind="ExternalOutput").ap()

    with tile.TileContext(nc) as tc:
        _emit_body(tc, out_d, xT_d, wqkT_d, wvT_d, wprojT_d)

    nc.compile()
    return nc


def _get_program():
    if "nc" not in _CACHE:
        _CACHE["nc"] = _build_program()
    return _CACHE["nc"]


def _make_in_maps(x, w_qkv, w_proj):
    import ml_dtypes
    bf16 = ml_dtypes.bfloat16
    x = np.asarray(x, dtype=np.float32)
    w_qkv = np.asarray(w_qkv, dtype=np.float32)
    w_proj = np.asarray(w_proj, dtype=np.float32)
    xT = [np.ascontiguousarray(x[b].T).astype(bf16) for b in range(B)]
    in_maps = []
    for c in range(NCORES):
        b, hg = c // 4, c % 4
        rows = slice(hg * LQK, (hg + 1) * LQK)
        qk_rows = np.r_[np.arange(hg * LQK, (hg + 1) * LQK),
                        D + np.arange(hg * LQK, (hg + 1) * LQK)]
        in_maps.append({
            "xT": xT[b],
            "wqkT": np.ascontiguousarray(w_qkv[qk_rows, :].T).astype(bf16),
            "wvT": np.ascontiguousarray(
                w_qkv[2 * D + np.arange(hg * LQK, (hg + 1) * LQK), :].T).astype(bf16),
            "wprojT": np.ascontiguousarray(w_proj[:, rows].T).astype(bf16),
        })
    return in_maps


def kernel(x, w_qkv, w_proj, b_proj, _return_results=False, _trace=False):
    from concourse import bass_utils

    nc = _get_program()
    in_maps = _make_in_maps(x, w_qkv, w_proj)
    res = bass_utils.run_bass_kernel_spmd(
        nc, in_maps, list(range(NCORES)), trace=_trace)
    partials = np.stack([res.results[c]["out"] for c in range(NCORES)])
    out = partials.reshape(B, 4, N, D).sum(axis=1, dtype=np.float32)
    out = out + np.asarray(b_proj, dtype=np.float32)[None, None, :]
    out = out.astype(np.float32)
    if _return_results:
        return out, res
    return out


# revision 13
# speedup vs baseline: 1.2392x; 1.0040x over previous
"""Multi-head attention (B=2, N=2048, D=1024, H=16, hd=64) on 8 trn2 NeuronCores.

Sharding: 8 cores = 2 (batch) x 4 (head groups of 4 heads).
Core c: batch b = c // 4, heads hg*4 .. hg*4+3 where hg = c % 4.

Per-core program (identical SPMD program, per-core data):
  inputs (DRAM):
    xT     [1024, 2048]  = x[b].T
    wqkT   [1024, 512]   = w_qkv[[q rows, k rows] of local heads].T
    wvT    [1024, 256]   = w_qkv[v rows of local heads].T
    wprojT [256, 1024]   = w_proj[:, local head cols].T
  output:
    out    [2048, 1024]  partial (row-parallel) projection output

  1) qkT  [512, 2048]  = wqkT.T @ xT        (q,k in transposed layout)
     v    [2048, 256]  natural layout, stored per head with 64 ones
     columns appended (v_sb[:, st, h, 64:128] = 1) so the PV matmul
     replicates the softmax denominator across partitions 64..127.
  2) per (head, q-block 512): scores_T [keys, q] = kT.T @ qT, exp on ACT
     (scale 1/8 fused) -> probs bf16; PV with ones-augmented V gives
     psum [128, 512]: rows 0..63 unnormalized out_T, rows 64..127 the
     denominator replicated. reciprocal_approx_fast (DVE custom op) +
     one cross-partition tensor_mul normalizes straight into proj
     layout (ao_sb) - no PE broadcast matmul, no DMA hop.
  3) partial = attn_out_T.T @ wprojT -> [2048, 1024], DMA'd to DRAM
     directly from PSUM.

Host unshard: out[b] = sum over 4 head-group partials + b_proj.
"""

import sys

if "/opt/trn_rl_repo" not in sys.path:
    sys.path.insert(0, "/opt/trn_rl_repo")

import numpy as np

B, N, D, H, HD = 2, 2048, 1024, 16, 64
NCORES = 8
HPC = 4               # heads per core
LQK = HPC * HD        # 256 local q (or k) rows
SCALE = HD ** -0.5    # 0.125

_CACHE = {}


def _emit_body(tc, out_d, xT_d, wqkT_d, wvT_d, wprojT_d):
    from concourse import mybir

    F32 = mybir.dt.float32
    BF16 = mybir.dt.bfloat16
    Exp = mybir.ActivationFunctionType.Exp

    nc = tc.nc
    KT = D // 128        # 8 contraction tiles for qkv gemms
    NB = N // 512        # 4 seq blocks
    NT = N // 128        # 16 seq tiles

    if True:
        with (
            nc.allow_low_precision(reason="bf16 matmul operands"),
            tc.tile_pool(name="w", bufs=1) as wpool,
            tc.tile_pool(name="x", bufs=1) as xpool,
            tc.tile_pool(name="qk", bufs=1) as qkpool,
            tc.tile_pool(name="vaug", bufs=1) as vapool,
            tc.tile_pool(name="ao", bufs=1) as aopool,
            tc.tile_pool(name="probs", bufs=3) as prpool,
            tc.tile_pool(name="recip", bufs=2) as rcpool,
            tc.tile_pool(name="stage", bufs=2) as stpool,
            tc.tile_pool(name="sc", bufs=2, space="PSUM") as scpool,
            tc.tile_pool(name="small", bufs=2, space="PSUM") as smps,
            tc.tile_pool(name="pv", bufs=2, space="PSUM") as pvps,
        ):
            # ---- SBUF tiles ----
            x_sb = xpool.tile([128, KT, N], BF16)
            wqk_sb = wpool.tile([128, KT, 2 * LQK], BF16)
            wv_sb = wpool.tile([128, KT, LQK], BF16)
            wproj_sb = wpool.tile([128, 2, D], BF16)
            qk_sb = qkpool.tile([128, 4, N], BF16)
            v_sb = vapool.tile([128, NT, HPC, 128], BF16)
            ao_sb = aopool.tile([128, 2, N], BF16)

            # ones columns so PV replicates the denominator on rows 0..63
            # (custom-DVE recip requires its input at partition offset 0)
            nc.gpsimd.memset(v_sb[:, :, :, 0:HD], 1.0)

            # ---- input DMAs, chunked so compute starts early ----
            xT_r = xT_d.rearrange("(kt p) n -> p kt n", p=128)
            wqkT_r = wqkT_d.rearrange("(kt p) m -> p kt m", p=128)
            wvT_r = wvT_d.rearrange("(kt p) m -> p kt m", p=128)
            for kt in range(KT):
                nc.sync.dma_start(out=wqk_sb[:, kt, :], in_=wqkT_r[:, kt, :])
            for kt in range(KT):
                nc.sync.dma_start(out=x_sb[:, kt, 0:1024], in_=xT_r[:, kt, 0:1024])
            for kt in range(KT):
                nc.sync.dma_start(out=wv_sb[:, kt, :], in_=wvT_r[:, kt, :])
            for kt in range(KT):
                nc.sync.dma_start(out=x_sb[:, kt, 1024:2048],
                                  in_=xT_r[:, kt, 1024:2048])
            nc.sync.dma_start(
                out=wproj_sb[:, :, :],
                in_=wprojT_d.rearrange("(kt p) o -> p kt o", p=128))

            # ---- qkT = wqkT.T @ xT : [512, 2048] ----
            # qk_sb m-tile layout: m=0: q heads 0,1 / m=1: q heads 2,3
            #                      m=2: k heads 0,1 / m=3: k heads 2,3
            def qk_gemm(m, nbs):
                wcol = (m % 2) * 128 + (0 if m < 2 else 2 * LQK // 2)
                for nb in nbs:
                    ps = smps.tile([128, 512], F32, tag="small")
                    for kt in range(KT):
                        nc.tensor.matmul(
                            ps[:, :],
                            wqk_sb[:, kt, wcol:wcol + 128],
                            x_sb[:, kt, nb * 512:(nb + 1) * 512],
                            start=(kt == 0), stop=(kt == KT - 1),
                        )
                    nc.vector.tensor_copy(
                        qk_sb[:, m, nb * 512:(nb + 1) * 512], ps[:, :])

            # ---- v natural [2048, 256]: st pairs -> [128, st, h, 0:64] ----
            def v_gemm(g):               # g = 0..7, st pair (2g, 2g+1)
                ps = smps.tile([128, 512], F32, tag="small")
                for i in range(2):
                    st = 2 * g + i
                    for kt in range(KT):
                        nc.tensor.matmul(
                            ps[:, i * 256:(i + 1) * 256],
                            x_sb[:, kt, st * 128:(st + 1) * 128],
                            wv_sb[:, kt, :],
                            start=(kt == 0), stop=(kt == KT - 1),
                        )
                nc.vector.tensor_copy(
                    v_sb[:, 2 * g:2 * g + 2, :, HD:128],
                    ps[:, :].rearrange("p (s h d) -> p s h d", s=2, h=HPC))

            # critical prefix, interleaved so chain (h0, qb0) starts after
            # just the first two groups and stays fed at the ACT exp pace
            qk_gemm(2, [0])
            qk_gemm(0, [0])
            v_gemm(0)
            qk_gemm(2, [1])
            v_gemm(1)
            qk_gemm(2, [2])
            v_gemm(2)
            qk_gemm(2, [3])
            v_gemm(3)
            qk_gemm(3, [0])
            v_gemm(4)
            qk_gemm(3, [1])
            v_gemm(5)
            qk_gemm(3, [2])
            v_gemm(6)
            qk_gemm(3, [3])
            v_gemm(7)
            qk_gemm(1, [0])

            # ---- attention: scores -> exp -> PV -> normalize ----
            def attn_chain(h, qb):
                pi = (h % 2) * 64
                mq, mk = h // 2, 2 + h // 2
                qT = qk_sb[pi:pi + 64, mq, qb * 512:(qb + 1) * 512]
                pv = pvps.tile([128, 512], F32, tag="pv")
                for kk in range(KT):          # pairs of key tiles
                    sc = scpool.tile([128, 1024], F32, tag="sc")
                    pr = prpool.tile([128, 1024], BF16, tag="probs")
                    for j in range(2):
                        kt = 2 * kk + j
                        kT = qk_sb[pi:pi + 64, mk, kt * 128:(kt + 1) * 128]
                        nc.tensor.matmul(
                            sc[:, j * 512:(j + 1) * 512], kT, qT,
                            start=True, stop=True)
                    nc.scalar.activation(pr[:, :], sc[:, :], Exp, scale=SCALE)
                    for j in range(2):
                        kt = 2 * kk + j
                        nc.tensor.matmul(
                            pv[:, :],
                            v_sb[:, kt, h, :],
                            pr[:, j * 512:(j + 1) * 512],
                            start=(kk == 0 and j == 0),
                            stop=(kk == KT - 1 and j == 1),
                        )
                recip = rcpool.tile([64, 512], F32, tag="recip")
                nc.vector.reciprocal_approx_fast(recip[:, :], pv[0:64, :])
                nc.vector.tensor_mul(
                    ao_sb[pi:pi + 64, h // 2, qb * 512:(qb + 1) * 512],
                    pv[64:128, :], recip[:, :])

            def proj(qb):
                for nt in range(qb * 4, qb * 4 + 4):
                    outst = stpool.tile([128, D], F32, tag="outst")
                    for ob in range(2):
                        ps = smps.tile([128, 512], F32, tag="small")
                        for kt2 in range(2):
                            nc.tensor.matmul(
                                ps[:, :],
                                ao_sb[:, kt2, nt * 128:(nt + 1) * 128],
                                wproj_sb[:, kt2, ob * 512:(ob + 1) * 512],
                                start=(kt2 == 0), stop=(kt2 == 1),
                            )
                        nc.vector.tensor_copy(
                            outst[:, ob * 512:(ob + 1) * 512], ps[:, :])
                    nc.sync.dma_start(
                        out=out_d[nt * 128:(nt + 1) * 128, :], in_=outst[:, :])

            # remaining q-gemms: deprioritized PE filler that slots into
            # the gaps of the ACT(exp)-paced attention chains
            with tc.high_priority(offset=-1000000):
                qk_gemm(0, [1])
                qk_gemm(1, [1])
                qk_gemm(0, [2])
                qk_gemm(1, [2])
                qk_gemm(0, [3])
                qk_gemm(1, [3])

            for qb in range(NB):
                for h in range(HPC):
                    attn_chain(h, qb)
                proj(qb)


def _build_program():
    import concourse.tile as tile
    from concourse import bacc, mybir

    F32 = mybir.dt.float32
    BF16 = mybir.dt.bfloat16

    nc = bacc.Bacc("TRN2", target_bir_lowering=False, debug=False,
                   num_devices=NCORES)

    xT_d = nc.dram_tensor("xT", [D, N], BF16, kind="ExternalInput").ap()
    wqkT_d = nc.dram_tensor("wqkT", [D, 2 * LQK], BF16, kind="ExternalInput").ap()
    wvT_d = nc.dram_tensor("wvT", [D, LQK], BF16, kind="ExternalInput").ap()
    wprojT_d = nc.dram_tensor("wprojT", [LQK, D], BF16, kind="ExternalInput").ap()
    out_d = nc.dram_tensor("out", [N, D], F32, kind="ExternalOutput").ap()

    with tile.TileContext(nc) as tc:
        _emit_body(tc, out_d, xT_d, wqkT_d, wvT_d, wprojT_d)

    nc.compile()
    return nc


def _get_program():
    if "nc" not in _CACHE:
        _CACHE["nc"] = _build_program()
    return _CACHE["nc"]


def _make_in_maps(x, w_qkv, w_proj):
    import ml_dtypes
    bf16 = ml_dtypes.bfloat16
    x = np.asarray(x, dtype=np.float32)
    w_qkv = np.asarray(w_qkv, dtype=np.float32)
    w_proj = np.asarray(w_proj, dtype=np.float32)
    xT = [np.ascontiguousarray(x[b].T).astype(bf16) for b in range(B)]
    in_maps = []
    for c in range(NCORES):
        b, hg = c // 4, c % 4
        rows = slice(hg * LQK, (hg + 1) * LQK)
        qk_rows = np.r_[np.arange(hg * LQK, (hg + 1) * LQK),
                        D + np.arange(hg * LQK, (hg + 1) * LQK)]
        in_maps.append({
            "xT": xT[b],
            "wqkT": np.ascontiguousarray(w_qkv[qk_rows, :].T).astype(bf16),
            "wvT": np.ascontiguousarray(
                w_qkv[2 * D + np.arange(hg * LQK, (hg + 1) * LQK), :].T).astype(bf16),
            "wprojT": np.ascontiguousarray(w_proj[:, rows].T).astype(bf16),
        })
    return in_maps


def kernel(x, w_qkv, w_proj, b_proj, _return_results=False, _trace=False):
    from concourse import bass_utils

    nc = _get_program()
    in_maps = _make_in_maps(x, w_qkv, w_proj)
    res = bass_utils.run_bass_kernel_spmd(
        nc, in_maps, list(range(NCORES)), trace=_trace)
    partials = np.stack([res.results[c]["out"] for c in range(NCORES)])
    out = partials.reshape(B, 4, N, D).sum(axis=1, dtype=np.float32)
    out = out + np.asarray(b_proj, dtype=np.float32)[None, None, :]
    out = out.astype(np.float32)
    if _return_results:
        return out, res
    return out
